# revision 8
# baseline (speedup 1.0000x reference)
"""Trainium2 Bass kernel for AssetSimilarityNetwork (pairwise-MLP similarity).

Computation (reference):
    proj = af @ Wp.T + bp                      # [N, 32]
    pa   = proj @ Wa.T  (Wa = W1[:, :32])      # [N, 32]
    pb   = proj @ Wb.T  (Wb = W1[:, 32:])      # [N, 32]
    h1   = relu(pa_i + pb_j + b1)              # per pair, 32
    h2   = relu(W2 @ h1 + b2)                  # per pair, 16
    sim  = sigmoid(w3 . h2 + b3)               # [N, N], diag forced to 1

Distribution: row-shard the N^2 grid over 8 NeuronCores (256 rows each).
Each core computes the full (tiny) projection locally; no collectives.

Per-core dataflow (bf16 compute, f32 accumulate):
  A : h1 = relu(pbT_rep4 + c_col)        DVE tensor_scalar (add, max) 4x bf16
  L2: h2 pre-act via 8 tile_position-packed matmuls (K=64, M=32) -> PSUM f32
  B : h2r = relu(psum + b2)              ACT/DVE split, writes bf16 SBUF
  L3: logits via j-dense matmuls: lhsT = h2r 128-col slice (stationary),
      rhs = W3 block-diag [128, 8]  -> PSUM [128 j, 8 i] (dense partitions)
  C : sigmoid(psum + b3) -> SBUF f32 -> DMA to HBM
"""

import sys
import types

import numpy as np

# ---------------------------------------------------------------- axon shim
sys.path.insert(0, "/root/.axon_site")
import antenv  # noqa: E402

if "antenv.axon_hooks" not in sys.modules:
    from trn_agent_boot.trn_boot import _ntff_profile_via_ctypes

    _mod = types.ModuleType("antenv.axon_hooks")
    try:
        _hook = _ntff_profile_via_ctypes("/opt/axon/libaxon_pjrt.so")
    except Exception:
        _hook = None
    _mod.get_axon_ntff_profile_hook = lambda: _hook
    _mod.set_axon_ntff_profile_hook = lambda h: None
    sys.modules["antenv.axon_hooks"] = _mod
    antenv.axon_hooks = _mod

import concourse.bass as bass  # noqa: E402
import concourse.tile as tile  # noqa: E402
from concourse import bacc, mybir  # noqa: E402
import concourse.bass_utils as bass_utils  # noqa: E402

bass_utils.upload_artifacts = lambda tmpdir: "(skipped)"
from concourse.bass_utils import run_bass_kernel_spmd  # noqa: E402

bf16 = mybir.dt.bfloat16
f32 = mybir.dt.float32
Alu = mybir.AluOpType
Act = mybir.ActivationFunctionType

N = 2048
FEAT = 64
NCORES = 8
ROWS = N // NCORES        # 256 i-rows per core
NST = ROWS // 16          # 16 super-tiles of 16 i's
NJT = N // 512            # 4 j-tiles of 512

_CACHE = {}


def _build_program():
    nc = bacc.Bacc()

    dp = nc.declare_dram_parameter
    afT = dp("afT", [FEAT + 1, N], f32, isOutput=False)       # af.T ; ones
    afs = dp("afs", [FEAT + 1, ROWS], f32, isOutput=False)    # slab af.T ; ones
    WpT = dp("WpT", [FEAT + 1, 32], f32, isOutput=False)      # Wp.T ; bp
    WaTb = dp("WaTb", [33, 128], f32, isOutput=False)         # (Wa.T;b1) tiled 4x cols
    WbT4 = dp("WbT4", [32, 128], f32, isOutput=False)         # Wb.T tiled 4x cols
    W2bd = dp("W2bd", [128, 32], f32, isOutput=False)         # L2 block-diag (2x stacked)
    W3bd = dp("W3bd", [128, 8], f32, isOutput=False)          # L3 block-diag
    b2col = dp("b2col", [128, 1], f32, isOutput=False)
    b3col = dp("b3col", [128, 1], f32, isOutput=False)
    out_d = dp("outT", [N, ROWS], f32, isOutput=True)  # transposed: [j, i]

    with tile.TileContext(nc, num_cores=NCORES) as tc:
        _build_body(nc, tc, afT, afs, WpT, WaTb, WbT4, W2bd, W3bd, b2col, b3col, out_d)
    nc.compile()
    return nc


def _build_body(nc, tc, afT, afs, WpT, WaTb, WbT4, W2bd, W3bd, b2col, b3col, out_d):
    from contextlib import ExitStack

    ctx = ExitStack()
    const = ctx.enter_context(tc.tile_pool(name="const", bufs=1))
    h1p = ctx.enter_context(tc.tile_pool(name="h1p", bufs=2))
    h2p = ctx.enter_context(tc.tile_pool(name="h2p", bufs=3))
    sigp = ctx.enter_context(tc.tile_pool(name="sigp", bufs=2))
    pre_ctx = ExitStack()
    psA = pre_ctx.enter_context(tc.tile_pool(name="psA", bufs=1, space="PSUM"))

    # ---------------- load + cast constants ----------------
    def load_bf(name, param, p, fdim):
        t = const.tile([p, fdim], f32, name=f"{name}_f")
        nc.sync.dma_start(t[:], param[:])
        tb = const.tile([p, fdim], bf16, name=f"{name}_b")
        nc.vector.tensor_copy(tb[:], t[:])
        return tb

    afT_bf = load_bf("afT", afT, FEAT + 1, N)
    afs_bf = load_bf("afs", afs, FEAT + 1, ROWS)
    WpT_bf = load_bf("WpT", WpT, FEAT + 1, 32)
    WaTb_bf = load_bf("WaTb", WaTb, 33, 128)
    WbT4_bf = load_bf("WbT4", WbT4, 32, 128)
    W2bd_bf = load_bf("W2bd", W2bd, 128, 32)
    W3bd_bf = load_bf("W3bd", W3bd, 128, 8)
    b2c = const.tile([128, 1], f32)
    nc.sync.dma_start(b2c[:], b2col[:])
    b3c = const.tile([128, 1], f32)
    nc.sync.dma_start(b3c[:], b3col[:])

    # ---------------- projection (full, for pb) ----------------
    projT_bf = const.tile([32, N], bf16)
    for c4 in range(4):
        pt = psA.tile([32, 512], f32, name=f"prj{c4}", tag="prj")
        nc.tensor.matmul(pt[:], WpT_bf[:], afT_bf[:, 512 * c4 : 512 * (c4 + 1)],
                         start=True, stop=True)
        nc.vector.tensor_copy(projT_bf[:, 512 * c4 : 512 * (c4 + 1)], pt[:])

    # pbT replicated 4x in partitions: lhsT = WbT4 [32, 128]
    pbT_rep4 = const.tile([128, N], bf16)
    for c4 in range(4):
        pt = psA.tile([128, 512], f32, name=f"pb{c4}", tag="pb")
        nc.tensor.matmul(pt[:], WbT4_bf[:], projT_bf[:, 512 * c4 : 512 * (c4 + 1)],
                         start=True, stop=True)
        nc.scalar.activation(pbT_rep4[:, 512 * c4 : 512 * (c4 + 1)], pt[:], Act.Copy)

    # ---------------- slab projection (for pa) ----------------
    projTs_aug = const.tile([33, ROWS], bf16)
    pts = psA.tile([32, ROWS], f32, name="prjs", tag="prj")
    nc.tensor.matmul(pts[:], WpT_bf[:], afs_bf[:], start=True, stop=True)
    nc.vector.tensor_copy(projTs_aug[:32, :], pts[:])
    nc.gpsimd.memset(projTs_aug[32:33, :], 1.0)

    # paT (+b1) replicated 4x in partitions: [128, ROWS] f32
    paT_sb = const.tile([128, ROWS], f32)
    pap = psA.tile([128, ROWS], f32, name="pap", tag="pap")
    nc.tensor.matmul(pap[:], WaTb_bf[:], projTs_aug[:], start=True, stop=True)
    nc.vector.tensor_copy(paT_sb[:], pap[:])

    # c_cols [128, NST*4]: column q = ST*4 + c holds pa(+b1) for the 4 i's
    # (R, a) at partition blocks b = 2R + a; i_local = ST*16 + R*8 + 2c + a.
    c_cols = const.tile([128, NST * 4], f32)
    paT_v = paT_sb[:].rearrange("p (st ii) -> p st ii", ii=16)
    cc_v = c_cols[:].rearrange("p (st c) -> p st c", c=4)
    for b in range(4):
        R, a = b // 2, b % 2
        src = paT_v[32 * b : 32 * b + 32, :, R * 8 + a : R * 8 + 8 : 2]
        dst = cc_v[32 * b : 32 * b + 32, :, :]
        nc.vector.tensor_copy(dst, src)

    # ---------------- main loop ----------------
    pre_ctx.close()  # release preamble PSUM pool
    psB = ctx.enter_context(tc.tile_pool(name="psB", bufs=2, space="PSUM"))
    psL = ctx.enter_context(tc.tile_pool(name="psL", bufs=2, space="PSUM"))
    # logits psum bank: 8 units of 64 slot-cols; unit u = (ST, jt) mod 8
    logits_ps = None
    sig_sb = None
    for ST in range(NST):
        # A-pass: h1_ST [128, 4c x 2048j] bf16
        h1_ST = h1p.tile([128, 4 * N], bf16, name=f"h1_{ST}", tag="h1")
        for c in range(4):
            nc.vector.tensor_scalar(
                h1_ST[:, N * c : N * (c + 1)],
                pbT_rep4[:],
                c_cols[:, ST * 4 + c : ST * 4 + c + 1],
                0.0,
                Alu.add,
                Alu.max,
            )
        for jt in range(NJT):
            u_abs = ST * NJT + jt
            u = u_abs % 8
            if u == 0:
                logits_ps = psL.tile([128, 512], f32, name=f"lg{u_abs}", tag="lg")
                sig_sb = sigp.tile([128, 512], f32, name=f"sg{u_abs}", tag="sg")

            # L2: 8 packed matmuls -> 2 psum tiles (R), [128, 512] f32
            h2r = []
            for R in range(2):
                ps = psB.tile([128, 512], f32, name=f"l2_{u_abs}_{R}", tag=f"l2_{R}")
                for c in range(4):
                    nc.tensor.matmul(
                        ps[32 * c : 32 * c + 32, :],
                        W2bd_bf[64 * R : 64 * R + 64, :],
                        h1_ST[64 * R : 64 * R + 64, N * c + 512 * jt : N * c + 512 * (jt + 1)],
                        start=True,
                        stop=True,
                        tile_position=(64 * R, 32 * c),
                    )
                # B-pass: relu(psum + b2) -> bf16; split engines by R
                hr = h2p.tile([128, 512], bf16, name=f"h2r_{u_abs}_{R}", tag=f"h2r_{R}")
                if R == 0:
                    nc.scalar.activation(hr[:], ps[:], Act.Relu, bias=b2c[:])
                else:
                    nc.vector.tensor_scalar(hr[:], ps[:], b2c[:], 0.0, Alu.add, Alu.max)
                h2r.append(hr)

            # L3: j-dense; lhsT = h2r 128-col slices, rhs = W3bd [128, 8]
            # slot layout within unit: s*16 + R*8 + m  (i = ST*16 + R*8 + m)
            for R in range(2):
                for s in range(4):
                    off = u * 64 + s * 16 + R * 8
                    nc.tensor.matmul(
                        logits_ps[:, off : off + 8],
                        h2r[R][:, 128 * s : 128 * (s + 1)],
                        W3bd_bf[:],
                        start=True,
                        stop=True,
                    )

            if u == 7:
                # C: sigmoid over the full bank, then DMA out per unit
                nc.scalar.activation(sig_sb[:], logits_ps[:], Act.Sigmoid, bias=b3c[:])
                for uu in range(8):
                    uab = u_abs - 7 + uu
                    ST2, jt2 = uab // NJT, uab % NJT
                    sb_v = sig_sb[:, uu * 64 : (uu + 1) * 64].rearrange(
                        "jp (s i) -> jp s i", s=4
                    )
                    dr_v = out_d[
                        jt2 * 512 : (jt2 + 1) * 512, ST2 * 16 : (ST2 + 1) * 16
                    ].rearrange("(s jp) i -> jp s i", s=4)
                    nc.sync.dma_start(dr_v, sb_v)
    ctx.close()


def _host_inputs(asset_features, Wp, bp, W1, b1, W2, b2, W3, b3, core):
    af = np.asarray(asset_features, np.float32)
    ones_n = np.ones((1, N), np.float32)
    ones_r = np.ones((1, ROWS), np.float32)
    sl = slice(core * ROWS, (core + 1) * ROWS)

    Wa = W1[:, :32]  # [32k, 32f]
    Wb = W1[:, 32:]

    WaTb_base = np.concatenate([Wa.T, b1[None, :]], axis=0)          # [33, 32]
    WbT4 = np.tile(Wb.T, (1, 4)).astype(np.float32)                  # [32, 128]
    WaTb = np.tile(WaTb_base, (1, 4)).astype(np.float32)             # [33, 128]

    # L2 block-diag [64, 32]: rows 32a+k, cols 16a+h = W2[h, k]
    W2bd64 = np.zeros((64, 32), np.float32)
    for a in range(2):
        W2bd64[32 * a : 32 * a + 32, 16 * a : 16 * a + 16] = W2.T    # [k, h]
    W2bd = np.tile(W2bd64, (2, 1))                                   # [128, 32]

    # L3 block-diag [128, 8]: rows 32c+16a+h, col m = 2c+a -> w3[h]
    W3bd = np.zeros((128, 8), np.float32)
    for c in range(4):
        for a in range(2):
            W3bd[32 * c + 16 * a : 32 * c + 16 * a + 16, 2 * c + a] = W3[0]

    b2col = np.tile(b2, 8).reshape(128, 1).astype(np.float32)
    b3col = np.full((128, 1), b3[0], np.float32)

    return {
        "afT": np.ascontiguousarray(np.concatenate([af.T, ones_n], axis=0)),
        "afs": np.ascontiguousarray(
            np.concatenate([af.T[:, sl], ones_r], axis=0)
        ),
        "WpT": np.concatenate([Wp.T, bp[None, :]], axis=0).astype(np.float32),
        "WaTb": WaTb,
        "WbT4": WbT4,
        "W2bd": W2bd,
        "W3bd": W3bd,
        "b2col": b2col,
        "b3col": b3col,
    }


def kernel(asset_features, Wp, bp, W1, b1, W2, b2, W3, b3, _trace=False):
    if "nc" not in _CACHE:
        _CACHE["nc"] = _build_program()
    nc = _CACHE["nc"]

    in_maps = [
        _host_inputs(asset_features, Wp, bp, W1, b1, W2, b2, W3, b3, core)
        for core in range(NCORES)
    ]
    res = run_bass_kernel_spmd(nc, in_maps, list(range(NCORES)), trace=_trace)
    _CACHE["last_exec_time_ns"] = res.exec_time_ns

    out = np.empty((N, N), np.float32)
    for c in range(NCORES):
        out[c * ROWS : (c + 1) * ROWS, :] = res.results[c]["outT"].T
    np.fill_diagonal(out, 1.0)
    return out
